# revision 10
# baseline (speedup 1.0000x reference)
"""Trainium2 Bass kernel for nn_OmegaEntangle (E^T C E with entangle coefficients).

Math (validated vs reference to ~8e-7 rel err in fp32):
  p_i = sum_j v_ij^2 ; m_i = mean_j v_ij
  C[i,j] = mask(i<j) * sqrt(p_i p_j) * (m_i + 1j*m_j) / sqrt(m_i^2 + m_j^2)
  out = E^T C E   (complex, E real)  ->  out_re = E^T Cr E, out_im = E^T Ci E

Sharding: data-parallel over the 2048 OUTPUT COLUMNS (256 per core), with the
p/m reduction row-sharded (64 rows per core).

Two NEFF launches (a device collective would cost ~60+ us of entry-barrier +
AllGather latency on this platform for 768 bytes; host concat of the tiny
reduction result is far cheaper):
  Kernel A: each core reduces its [64, 32768] vuln shard -> p[64], msum[64].
  Host: concatenates the 8 shards (pure data movement, no math).
  Kernel B: each core derives sp/a/m2 vectors, builds C^T, computes
    T = C @ E[:, cols] and out[:, cols] = E^T @ T, writes [2048, 256] slabs.
Host concatenates slabs along columns -> [2048, 2048] complex64.
"""

import numpy as np

import concourse.bass as bass
import concourse.mybir as mybir
import concourse.tile as tile
from concourse import bacc
from concourse.bass_utils import run_bass_kernel_spmd

D = 512          # number of domains
V = 32768        # vuln dim
S = 2048         # sup (embed) dim
NCORES = 8
ROWS_PER_CORE = D // NCORES          # 64
COLS_PER_CORE = S // NCORES          # 256
NVT = 8                               # number of vuln tiles per core
VFREE = (ROWS_PER_CORE * V) // (128 * NVT)   # 2048 free elems per vuln tile
KT = D // 128                         # 4 contraction tiles
MT = S // 128                         # 16 output row tiles
INV_V = 1.0 / V
WARMUP_MMS = 14                       # PE warm-up matmuls at kernel-B start

F32 = mybir.dt.float32
F32R = mybir.dt.float32r
BF16 = mybir.dt.bfloat16
# float32r (TF32) matmul inputs stream at 1 cyc/row vs 4 for float32.
# Host pre-rounds E to TF32 values; on-device producers of matmul operands
# write float32r-typed tiles so the BIR verifier sees rounded inputs.


def _tf32_round(x):
    xi = np.ascontiguousarray(x, dtype=np.float32).view(np.uint32)
    return ((xi + np.uint32(0x1000)) & np.uint32(0xFFFFE000)).view(np.float32)
AF = mybir.ActivationFunctionType
ALU = mybir.AluOpType

_CACHE = {}


def build_kernel_a():
    """Reduce kernel: per-core p/msum over the 64-row vuln shard."""
    nc = bacc.Bacc("TRN2", target_bir_lowering=False, debug=False, num_devices=NCORES)

    v128 = nc.dram_tensor("v128", [128, NVT, VFREE], F32, kind="ExternalInput")
    pairmat = nc.dram_tensor("pairmat", [128, ROWS_PER_CORE], F32, kind="ExternalInput")
    out_pm = nc.dram_tensor("out_pm", [ROWS_PER_CORE, 2], F32, kind="ExternalOutput")

    with tile.TileContext(nc) as tc:
        with (
            tc.tile_pool(name="vin", bufs=3) as vin_pool,
            tc.tile_pool(name="scr", bufs=2) as scr_pool,
            tc.tile_pool(name="small", bufs=1) as small_pool,
            tc.tile_pool(name="ps", bufs=1, space="PSUM") as ps_pool,
        ):
            vts = []
            for t in range(NVT):
                vt = vin_pool.tile([128, VFREE], F32, name=f"vt{t}", tag="vt")
                eng = nc.sync if t % 2 == 0 else nc.scalar
                eng.dma_start(vt[:], v128[:, t, :])
                vts.append(vt)
            pair_sb = small_pool.tile([128, ROWS_PER_CORE], F32, name="pair_sb")
            nc.sync.dma_start(pair_sb[:], pairmat[:])

            pm_acc = small_pool.tile([128, 2 * NVT], F32, name="pm_acc")
            for t in range(NVT):
                sq = scr_pool.tile([128, VFREE], F32, name="sq", tag="sq")
                nc.scalar.activation(
                    sq[:], vts[t][:], AF.Square, accum_out=pm_acc[:, t : t + 1]
                )
                raw = scr_pool.tile([128, VFREE], F32, name="raw", tag="raw")
                nc.vector.tensor_scalar(
                    raw[:], vts[t][:], 1.0, None, ALU.mult, ALU.add,
                    accum_out=pm_acc[:, NVT + t : NVT + t + 1],
                )

            ps_pm = ps_pool.tile([ROWS_PER_CORE, 2 * NVT], F32, name="ps_pm")
            nc.tensor.matmul(ps_pm[:], pair_sb[:], pm_acc[:], start=True, stop=True)

            d2 = small_pool.tile([ROWS_PER_CORE, 2], F32, name="d2")
            nc.vector.tensor_reduce(
                d2[:, 0:1], ps_pm[:, 0:NVT], mybir.AxisListType.X, ALU.add
            )
            nc.vector.tensor_reduce(
                d2[:, 1:2], ps_pm[:, NVT : 2 * NVT], mybir.AxisListType.X, ALU.add
            )
            nc.sync.dma_start(out_pm[:], d2[:])

    nc.compile()
    return nc


def build_kernel_b():
    """Main kernel: derive vectors, build C^T, two matmul chains, write slab."""
    nc = bacc.Bacc("TRN2", target_bir_lowering=False, debug=False, num_devices=NCORES)

    # pm_pp: per-partition layout, col kt   = p[q + 128*kt],
    #        col 4+kt = msum[q + 128*kt]    (q = partition)
    pm_pp = nc.dram_tensor("pm_pp", [128, 2 * KT], F32, kind="ExternalInput")
    # row layouts (ordered 0..511), each on a single partition
    p_row_in = nc.dram_tensor("p_row", [1, D], F32, kind="ExternalInput")
    ms_row_in = nc.dram_tensor("ms_row", [1, D], F32, kind="ExternalInput")
    efull = nc.dram_tensor("efull", [KT, 128, S], F32R, kind="ExternalInput")
    ecols = nc.dram_tensor("ecols", [KT, 128, COLS_PER_CORE], F32R, kind="ExternalInput")
    # transposed output slabs: host transposes back (out[:, cols] = slab.T)
    out_re = nc.dram_tensor("out_re", [COLS_PER_CORE, S], F32, kind="ExternalOutput")
    out_im = nc.dram_tensor("out_im", [COLS_PER_CORE, S], F32, kind="ExternalOutput")

    with tile.TileContext(nc) as tc:
        with (
            tc.tile_pool(name="epool", bufs=1) as e_pool,
            tc.tile_pool(name="small", bufs=1) as small_pool,
            tc.tile_pool(name="cbuild", bufs=2) as cb_pool,
            tc.tile_pool(name="ctp", bufs=1) as ct_pool,
            tc.tile_pool(name="tsb", bufs=1) as t_pool,
            tc.tile_pool(name="ost", bufs=4) as o_pool,
            tc.tile_pool(name="psA", bufs=4, space="PSUM") as psA,
            tc.tile_pool(name="psB", bufs=4, space="PSUM") as psB,
        ):
            # -------- input DMAs (small first, then E) ------------------------
            pp = small_pool.tile([128, 2 * KT], F32, name="pp")
            nc.sync.dma_start(pp[:], pm_pp[:])
            prow = small_pool.tile([1, D], F32, name="prow")
            nc.sync.dma_start(prow[:], p_row_in[:])
            msrow = small_pool.tile([1, D], F32, name="msrow")
            nc.sync.dma_start(msrow[:], ms_row_in[:])

            ec_sb = []
            for kt in range(KT):
                ect = e_pool.tile(
                    [128, COLS_PER_CORE], F32R, name=f"ec{kt}", tag=f"ec{kt}"
                )
                nc.sync.dma_start(ect[:], ecols[kt])
                ec_sb.append(ect)
            e_sb = []
            for kt in range(KT):
                et = e_pool.tile([128, S], F32R, name=f"e{kt}", tag=f"e{kt}")
                nc.sync.dma_start(et[:], efull[kt])
                e_sb.append(et)

            # -------- PE warm-up during the small-vector derivation -----------
            ones_sb = small_pool.tile([1, 128], F32, name="ones_sb")
            nc.vector.memset(ones_sb[:], 1.0)
            warm_b = small_pool.tile([128, 128], BF16, name="warm_b")
            nc.gpsimd.memset(warm_b[:], 0.001)
            ps_w = psA.tile([128, 512], F32, name="ps_w", tag="t")
            for i in range(WARMUP_MMS):
                nc.tensor.matmul(
                    ps_w[:, 0:128], warm_b[:], warm_b[:],
                    start=(i == 0), stop=(i == WARMUP_MMS - 1),
                )

            # -------- derived vectors -----------------------------------------
            # per-partition [128, 4] each
            sp4 = small_pool.tile([128, KT], F32, name="sp4")
            a4 = small_pool.tile([128, KT], F32, name="a4")
            m24 = small_pool.tile([128, KT], F32, name="m24")
            nc.scalar.activation(sp4[:], pp[:, 0:KT], AF.Sqrt)
            nc.vector.scalar_tensor_tensor(
                a4[:], pp[:, KT : 2 * KT], INV_V, sp4[:], op0=ALU.mult, op1=ALU.mult
            )
            nc.vector.scalar_tensor_tensor(
                m24[:], pp[:, KT : 2 * KT], INV_V * INV_V, pp[:, KT : 2 * KT],
                op0=ALU.mult, op1=ALU.mult,
            )
            # row layout [1, 512] each
            sp_row = small_pool.tile([1, D], F32, name="sp_row")
            a_row = small_pool.tile([1, D], F32, name="a_row")
            m2_row = small_pool.tile([1, D], F32, name="m2_row")
            nc.scalar.activation(sp_row[:], prow[:], AF.Sqrt)
            nc.vector.scalar_tensor_tensor(
                a_row[:], msrow[:], INV_V, sp_row[:], op0=ALU.mult, op1=ALU.mult
            )
            nc.vector.scalar_tensor_tensor(
                m2_row[:], msrow[:], INV_V * INV_V, msrow[:],
                op0=ALU.mult, op1=ALU.mult,
            )

            # broadcast rows to 128 partitions via K=1 matmuls
            sp_bc = small_pool.tile([128, D], F32, name="sp_bc")
            a_bc = small_pool.tile([128, D], F32, name="a_bc")
            m2_bc = small_pool.tile([128, D], F32, name="m2_bc")
            for row, bc in ((sp_row, sp_bc), (a_row, a_bc), (m2_row, m2_bc)):
                ps_bc = psB.tile([128, D], F32, name="ps_bc", tag="o")
                nc.tensor.matmul(ps_bc[:], ones_sb[:], row[:], start=True, stop=True)
                nc.scalar.copy(bc[:], ps_bc[:])

            # -------- build C^T (real and imag) -------------------------------
            ct_r, ct_i = [], []
            for jt in range(KT):
                h = cb_pool.tile([128, D], F32, name="h", tag="h")
                nc.scalar.activation(
                    h[:], m2_bc[:], AF.Sqrt, bias=m24[:, jt : jt + 1], scale=1.0
                )
                rinv = cb_pool.tile([128, D], F32, name="rinv", tag="rinv")
                nc.vector.reciprocal_approx_fast(out=rinv[:], in_=h[:])
                rm = cb_pool.tile([128, D], F32, name="rm", tag="rm")
                nc.gpsimd.affine_select(
                    out=rm[:], in_=rinv[:],
                    pattern=[[-1, D]], compare_op=ALU.is_gt,
                    fill=0.0, base=128 * jt, channel_multiplier=1,
                )
                ctr = ct_pool.tile([128, D], F32R, name=f"ctr{jt}", tag=f"ctr{jt}")
                cti = ct_pool.tile([128, D], F32R, name=f"cti{jt}", tag=f"cti{jt}")
                nc.vector.scalar_tensor_tensor(
                    ctr[:], a_bc[:], sp4[:, jt : jt + 1], rm[:],
                    op0=ALU.mult, op1=ALU.mult,
                )
                nc.vector.scalar_tensor_tensor(
                    cti[:], sp_bc[:], a4[:, jt : jt + 1], rm[:],
                    op0=ALU.mult, op1=ALU.mult,
                )
                ct_r.append(ctr)
                ct_i.append(cti)

            # -------- T = C @ E[:, cols]  ([128, 512] = [T_r | T_i]) ----------
            t_sb = []
            for it in range(KT):
                ps_t = psA.tile([128, 2 * COLS_PER_CORE], F32, name="ps_t", tag="t")
                for jt in range(KT):
                    nc.tensor.matmul(
                        ps_t[:, 0:COLS_PER_CORE],
                        ct_r[jt][:, it * 128 : (it + 1) * 128],
                        ec_sb[jt][:],
                        start=(jt == 0), stop=(jt == KT - 1),
                    )
                for jt in range(KT):
                    nc.tensor.matmul(
                        ps_t[:, COLS_PER_CORE : 2 * COLS_PER_CORE],
                        ct_i[jt][:, it * 128 : (it + 1) * 128],
                        ec_sb[jt][:],
                        start=(jt == 0), stop=(jt == KT - 1),
                    )
                tsb = t_pool.tile(
                    [128, 2 * COLS_PER_CORE], F32R, name=f"tsb{it}", tag=f"tsb{it}"
                )
                if it % 2 == 0:
                    nc.scalar.copy(tsb[:], ps_t[:])
                else:
                    nc.vector.tensor_copy(tsb[:], ps_t[:])
                t_sb.append(tsb)

            # -------- out^T[cols, :] = T^T @ E  (transposed chain) ------------
            # lhsT = T[i, c] slices straight from t_sb; rhs = e_sb 512-chunks.
            # Consecutive sn-matmuls share the same stationary operand.
            NS = S // 512
            cnt = 0
            for part, outT in ((0, out_re), (1, out_im)):
                for mc in range(2):
                    c0 = part * COLS_PER_CORE + mc * 128
                    pso = [
                        psB.tile([128, 512], F32, name=f"pso{sn}", tag="o")
                        for sn in range(NS)
                    ]
                    for it in range(KT):
                        for sn in range(NS):
                            nc.tensor.matmul(
                                pso[sn][:],
                                t_sb[it][:, c0 : c0 + 128],
                                e_sb[it][:, sn * 512 : (sn + 1) * 512],
                                start=(it == 0), stop=(it == KT - 1),
                            )
                    for sn in range(NS):
                        osb = o_pool.tile([128, 512], F32, name="osb", tag="osb")
                        if cnt % 2 == 0:
                            nc.scalar.copy(osb[:], pso[sn][:])
                        else:
                            nc.vector.tensor_copy(osb[:], pso[sn][:])
                        eng = nc.sync if cnt % 2 == 0 else nc.scalar
                        eng.dma_start(
                            outT[mc * 128 : (mc + 1) * 128, sn * 512 : (sn + 1) * 512],
                            osb[:],
                        )
                        cnt += 1

    nc.compile()
    return nc


def _prepare_a_in_maps(vulns):
    vulns = np.ascontiguousarray(np.asarray(vulns, dtype=np.float32))
    pair = np.ascontiguousarray(
        np.repeat(np.eye(ROWS_PER_CORE, dtype=np.float32), 2, axis=0)
    )
    in_maps = []
    for c in range(NCORES):
        vsh = vulns[c * ROWS_PER_CORE : (c + 1) * ROWS_PER_CORE]
        in_maps.append(
            {
                "v128": np.ascontiguousarray(vsh.reshape(128, NVT, VFREE)),
                "pairmat": pair,
            }
        )
    return in_maps


def _prepare_b_in_maps(embed_table, domain_ids, p_full, msum_full):
    embed_table = np.ascontiguousarray(np.asarray(embed_table, dtype=np.float32))
    domain_ids = np.asarray(domain_ids).astype(np.int64)
    E = np.ascontiguousarray(embed_table[domain_ids])  # [512, 2048]
    e4 = _tf32_round(E).reshape(KT, 128, S)
    # per-partition layout [128, 8]
    pm_pp = np.empty((128, 2 * KT), dtype=np.float32)
    pm_pp[:, 0:KT] = p_full.reshape(KT, 128).T
    pm_pp[:, KT : 2 * KT] = msum_full.reshape(KT, 128).T
    p_row = np.ascontiguousarray(p_full.astype(np.float32).reshape(1, D))
    ms_row = np.ascontiguousarray(msum_full.astype(np.float32).reshape(1, D))
    in_maps = []
    for c in range(NCORES):
        in_maps.append(
            {
                "pm_pp": pm_pp,
                "p_row": p_row,
                "ms_row": ms_row,
                "efull": e4,
                "ecols": np.ascontiguousarray(
                    e4[:, :, c * COLS_PER_CORE : (c + 1) * COLS_PER_CORE]
                ),
            }
        )
    return in_maps


def kernel(vulns, embed_table, domain_ids, _trace=False):
    if "nc_a" not in _CACHE:
        _CACHE["nc_a"] = build_kernel_a()
    if "nc_b" not in _CACHE:
        _CACHE["nc_b"] = build_kernel_b()

    res_a = run_bass_kernel_spmd(
        _CACHE["nc_a"], _prepare_a_in_maps(vulns),
        core_ids=list(range(NCORES)), trace=_trace,
    )
    _CACHE["res_a"] = res_a
    p_full = np.concatenate([res_a.results[c]["out_pm"][:, 0] for c in range(NCORES)])
    msum_full = np.concatenate(
        [res_a.results[c]["out_pm"][:, 1] for c in range(NCORES)]
    )

    res_b = run_bass_kernel_spmd(
        _CACHE["nc_b"], _prepare_b_in_maps(embed_table, domain_ids, p_full, msum_full),
        core_ids=list(range(NCORES)), trace=_trace,
    )
    _CACHE["res_b"] = res_b

    out = np.empty((S, S), dtype=np.complex64)
    for c in range(NCORES):
        r = res_b.results[c]
        sl = slice(c * COLS_PER_CORE, (c + 1) * COLS_PER_CORE)
        out[:, sl] = r["out_re"].T + 1j * r["out_im"].T
    return out


if __name__ == "__main__":
    rng = np.random.default_rng(0)
    v = rng.standard_normal((D, V), dtype=np.float32)
    et = rng.standard_normal((D, S), dtype=np.float32)
    ids = np.arange(D, dtype=np.int32)
    out = kernel(v, et, ids)
    print(out.shape, out.dtype)


# revision 13
# speedup vs baseline: 1.0139x; 1.0139x over previous
"""Trainium2 Bass kernel for nn_OmegaEntangle (E^T C E with entangle coefficients).

Math (validated vs reference to ~8e-7 rel err in fp32):
  p_i = sum_j v_ij^2 ; m_i = mean_j v_ij
  C[i,j] = mask(i<j) * sqrt(p_i p_j) * (m_i + 1j*m_j) / sqrt(m_i^2 + m_j^2)
  out = E^T C E   (complex, E real)  ->  out_re = E^T Cr E, out_im = E^T Ci E

Sharding: data-parallel over the 2048 OUTPUT COLUMNS (256 per core), with the
p/m reduction row-sharded (64 rows per core).

Two NEFF launches (a device collective would cost ~60+ us of entry-barrier +
AllGather latency on this platform for 768 bytes; host concat of the tiny
reduction result is far cheaper):
  Kernel A: each core reduces its [64, 32768] vuln shard -> p[64], msum[64].
  Host: concatenates the 8 shards (pure data movement, no math).
  Kernel B: each core derives sp/a/m2 vectors, builds C^T, computes
    T = C @ E[:, cols] and out[:, cols] = E^T @ T, writes [2048, 256] slabs.
Host concatenates slabs along columns -> [2048, 2048] complex64.
"""

import numpy as np

import concourse.bass as bass
import concourse.mybir as mybir
import concourse.tile as tile
from concourse import bacc
from concourse.bass_utils import run_bass_kernel_spmd

D = 512          # number of domains
V = 32768        # vuln dim
S = 2048         # sup (embed) dim
NCORES = 8
ROWS_PER_CORE = D // NCORES          # 64
COLS_PER_CORE = S // NCORES          # 256
NVT = 8                               # number of vuln tiles per core
VFREE = (ROWS_PER_CORE * V) // (128 * NVT)   # 2048 free elems per vuln tile
KT = D // 128                         # 4 contraction tiles
MT = S // 128                         # 16 output row tiles
INV_V = 1.0 / V
WARMUP_MMS = 30                       # PE warm-up matmuls at kernel-B start

F32 = mybir.dt.float32
F32R = mybir.dt.float32r
BF16 = mybir.dt.bfloat16
# float32r (TF32) matmul inputs stream at 1 cyc/row vs 4 for float32.
# Host pre-rounds E to TF32 values; on-device producers of matmul operands
# write float32r-typed tiles so the BIR verifier sees rounded inputs.


def _tf32_round(x):
    xi = np.ascontiguousarray(x, dtype=np.float32).view(np.uint32)
    return ((xi + np.uint32(0x1000)) & np.uint32(0xFFFFE000)).view(np.float32)
AF = mybir.ActivationFunctionType
ALU = mybir.AluOpType

_CACHE = {}


def build_kernel_a():
    """Reduce kernel: per-core p/msum over the 64-row vuln shard."""
    nc = bacc.Bacc("TRN2", target_bir_lowering=False, debug=False, num_devices=NCORES)

    v128 = nc.dram_tensor("v128", [128, NVT, VFREE], F32, kind="ExternalInput")
    pairmat = nc.dram_tensor("pairmat", [128, ROWS_PER_CORE], F32, kind="ExternalInput")
    out_pm = nc.dram_tensor("out_pm", [ROWS_PER_CORE, 2], F32, kind="ExternalOutput")

    with tile.TileContext(nc) as tc:
        with (
            tc.tile_pool(name="vin", bufs=3) as vin_pool,
            tc.tile_pool(name="scr", bufs=2) as scr_pool,
            tc.tile_pool(name="small", bufs=1) as small_pool,
            tc.tile_pool(name="ps", bufs=1, space="PSUM") as ps_pool,
        ):
            vts = []
            for t in range(NVT):
                vt = vin_pool.tile([128, VFREE], F32, name=f"vt{t}", tag="vt")
                nc.sync.dma_start(vt[:], v128[:, t, :])
                vts.append(vt)
            pair_sb = small_pool.tile([128, ROWS_PER_CORE], F32, name="pair_sb")
            nc.sync.dma_start(pair_sb[:], pairmat[:])

            pm_acc = small_pool.tile([128, 2 * NVT], F32, name="pm_acc")
            for t in range(NVT):
                sq = scr_pool.tile([128, VFREE], F32, name="sq", tag="sq")
                nc.scalar.activation(
                    sq[:], vts[t][:], AF.Square, accum_out=pm_acc[:, t : t + 1]
                )
                raw = scr_pool.tile([128, VFREE], F32, name="raw", tag="raw")
                nc.vector.tensor_scalar(
                    raw[:], vts[t][:], 1.0, None, ALU.mult, ALU.add,
                    accum_out=pm_acc[:, NVT + t : NVT + t + 1],
                )

            ps_pm = ps_pool.tile([ROWS_PER_CORE, 2 * NVT], F32, name="ps_pm")
            nc.tensor.matmul(ps_pm[:], pair_sb[:], pm_acc[:], start=True, stop=True)

            d2 = small_pool.tile([ROWS_PER_CORE, 2], F32, name="d2")
            nc.vector.tensor_reduce(
                d2[:, 0:1], ps_pm[:, 0:NVT], mybir.AxisListType.X, ALU.add
            )
            nc.vector.tensor_reduce(
                d2[:, 1:2], ps_pm[:, NVT : 2 * NVT], mybir.AxisListType.X, ALU.add
            )
            nc.sync.dma_start(out_pm[:], d2[:])

    nc.compile()
    return nc


def build_kernel_b():
    """Main kernel: derive vectors, build C^T, two matmul chains, write slab."""
    nc = bacc.Bacc("TRN2", target_bir_lowering=False, debug=False, num_devices=NCORES)

    # pm_pp: per-partition layout, col kt   = p[q + 128*kt],
    #        col 4+kt = msum[q + 128*kt]    (q = partition)
    pm_pp = nc.dram_tensor("pm_pp", [128, 2 * KT], F32, kind="ExternalInput")
    # row layouts (ordered 0..511), each on a single partition
    p_row_in = nc.dram_tensor("p_row", [1, D], F32, kind="ExternalInput")
    ms_row_in = nc.dram_tensor("ms_row", [1, D], F32, kind="ExternalInput")
    efull = nc.dram_tensor("efull", [KT, 128, S], F32R, kind="ExternalInput")
    ecols = nc.dram_tensor("ecols", [KT, 128, COLS_PER_CORE], F32R, kind="ExternalInput")
    # transposed output slabs: host transposes back (out[:, cols] = slab.T)
    out_re = nc.dram_tensor("out_re", [COLS_PER_CORE, S], F32, kind="ExternalOutput")
    out_im = nc.dram_tensor("out_im", [COLS_PER_CORE, S], F32, kind="ExternalOutput")

    with tile.TileContext(nc) as tc:
        with (
            tc.tile_pool(name="epool", bufs=1) as e_pool,
            tc.tile_pool(name="small", bufs=1) as small_pool,
            tc.tile_pool(name="cbuild", bufs=2) as cb_pool,
            tc.tile_pool(name="ctp", bufs=1) as ct_pool,
            tc.tile_pool(name="tsb", bufs=1) as t_pool,
            tc.tile_pool(name="ost", bufs=4) as o_pool,
            tc.tile_pool(name="psA", bufs=4, space="PSUM") as psA,
            tc.tile_pool(name="psB", bufs=4, space="PSUM") as psB,
        ):
            # -------- input DMAs (small first, then E) ------------------------
            pp = small_pool.tile([128, 2 * KT], F32, name="pp")
            nc.sync.dma_start(pp[:], pm_pp[:])
            prow = small_pool.tile([1, D], F32, name="prow")
            nc.sync.dma_start(prow[:], p_row_in[:])
            msrow = small_pool.tile([1, D], F32, name="msrow")
            nc.sync.dma_start(msrow[:], ms_row_in[:])

            ec_sb = []
            for kt in range(KT):
                ect = e_pool.tile(
                    [128, COLS_PER_CORE], F32R, name=f"ec{kt}", tag=f"ec{kt}"
                )
                nc.sync.dma_start(ect[:], ecols[kt])
                ec_sb.append(ect)
            e_sb = []
            for kt in range(KT):
                et = e_pool.tile([128, S], F32R, name=f"e{kt}", tag=f"e{kt}")
                nc.sync.dma_start(et[:], efull[kt])
                e_sb.append(et)

            # -------- PE warm-up during the small-vector derivation -----------
            ones_sb = small_pool.tile([1, 128], F32, name="ones_sb")
            nc.vector.memset(ones_sb[:], 1.0)
            warm_b = small_pool.tile([128, 512], BF16, name="warm_b")
            nc.gpsimd.memset(warm_b[:], 0.001)
            ps_w = psB.tile([128, 512], F32, name="ps_w", tag="o")
            for i in range(WARMUP_MMS):
                nc.tensor.matmul(
                    ps_w[:], warm_b[:, 0:128], warm_b[:],
                    start=(i == 0), stop=(i == WARMUP_MMS - 1),
                )

            # -------- derived vectors -----------------------------------------
            # per-partition [128, 4] each
            sp4 = small_pool.tile([128, KT], F32, name="sp4")
            a4 = small_pool.tile([128, KT], F32, name="a4")
            m24 = small_pool.tile([128, KT], F32, name="m24")
            nc.scalar.activation(sp4[:], pp[:, 0:KT], AF.Sqrt)
            nc.vector.scalar_tensor_tensor(
                a4[:], pp[:, KT : 2 * KT], INV_V, sp4[:], op0=ALU.mult, op1=ALU.mult
            )
            nc.vector.scalar_tensor_tensor(
                m24[:], pp[:, KT : 2 * KT], INV_V * INV_V, pp[:, KT : 2 * KT],
                op0=ALU.mult, op1=ALU.mult,
            )
            # row layout [1, 512] each
            sp_row = small_pool.tile([1, D], F32, name="sp_row")
            a_row = small_pool.tile([1, D], F32, name="a_row")
            m2_row = small_pool.tile([1, D], F32, name="m2_row")
            nc.scalar.activation(sp_row[:], prow[:], AF.Sqrt)
            nc.vector.scalar_tensor_tensor(
                a_row[:], msrow[:], INV_V, sp_row[:], op0=ALU.mult, op1=ALU.mult
            )
            nc.vector.scalar_tensor_tensor(
                m2_row[:], msrow[:], INV_V * INV_V, msrow[:],
                op0=ALU.mult, op1=ALU.mult,
            )

            # broadcast rows to 128 partitions via K=1 matmuls
            sp_bc = small_pool.tile([128, D], F32, name="sp_bc")
            a_bc = small_pool.tile([128, D], F32, name="a_bc")
            m2_bc = small_pool.tile([128, D], F32, name="m2_bc")
            for row, bc in ((sp_row, sp_bc), (a_row, a_bc), (m2_row, m2_bc)):
                ps_bc = psB.tile([128, D], F32, name="ps_bc", tag="o")
                nc.tensor.matmul(ps_bc[:], ones_sb[:], row[:], start=True, stop=True)
                nc.scalar.copy(bc[:], ps_bc[:])

            # -------- build C^T (real and imag) -------------------------------
            ct_r, ct_i = [], []
            for jt in range(KT):
                h = cb_pool.tile([128, D], F32, name="h", tag="h")
                nc.scalar.activation(
                    h[:], m2_bc[:], AF.Sqrt, bias=m24[:, jt : jt + 1], scale=1.0
                )
                rinv = cb_pool.tile([128, D], F32, name="rinv", tag="rinv")
                nc.vector.reciprocal_approx_fast(out=rinv[:], in_=h[:])
                rm = cb_pool.tile([128, D], F32, name="rm", tag="rm")
                nc.gpsimd.affine_select(
                    out=rm[:], in_=rinv[:],
                    pattern=[[-1, D]], compare_op=ALU.is_gt,
                    fill=0.0, base=128 * jt, channel_multiplier=1,
                )
                ctr = ct_pool.tile([128, D], F32R, name=f"ctr{jt}", tag=f"ctr{jt}")
                cti = ct_pool.tile([128, D], F32R, name=f"cti{jt}", tag=f"cti{jt}")
                nc.vector.scalar_tensor_tensor(
                    ctr[:], a_bc[:], sp4[:, jt : jt + 1], rm[:],
                    op0=ALU.mult, op1=ALU.mult,
                )
                nc.vector.scalar_tensor_tensor(
                    cti[:], sp_bc[:], a4[:, jt : jt + 1], rm[:],
                    op0=ALU.mult, op1=ALU.mult,
                )
                ct_r.append(ctr)
                ct_i.append(cti)

            # -------- T = C @ E[:, cols]  ([128, 512] = [T_r | T_i]) ----------
            ps_ts = [
                psA.tile(
                    [128, 2 * COLS_PER_CORE], F32, name=f"ps_t{it}", tag=f"t{it}",
                    bufs=1,
                )
                for it in range(KT)
            ]
            for part, cts in ((0, ct_r), (1, ct_i)):
                lo = part * COLS_PER_CORE
                for jt in range(KT):
                    for it in range(KT):
                        nc.tensor.matmul(
                            ps_ts[it][:, lo : lo + COLS_PER_CORE],
                            cts[jt][:, it * 128 : (it + 1) * 128],
                            ec_sb[jt][:],
                            start=(jt == 0), stop=(jt == KT - 1),
                        )
            t_sb = []
            for it in range(KT):
                tsb = t_pool.tile(
                    [128, 2 * COLS_PER_CORE], F32R, name=f"tsb{it}", tag=f"tsb{it}"
                )
                if it % 2 == 0:
                    nc.scalar.copy(tsb[:], ps_ts[it][:])
                else:
                    nc.vector.tensor_copy(tsb[:], ps_ts[it][:])
                t_sb.append(tsb)

            # -------- out^T[cols, :] = T^T @ E  (transposed chain) ------------
            # lhsT = T[i, c] slices straight from t_sb; rhs = e_sb 512-chunks.
            # Consecutive sn-matmuls share the same stationary operand.
            NS = S // 512
            cnt = 0
            for part, outT in ((0, out_re), (1, out_im)):
                for mc in range(2):
                    c0 = part * COLS_PER_CORE + mc * 128
                    pso = [
                        psB.tile([128, 512], F32, name=f"pso{sn}", tag="o")
                        for sn in range(NS)
                    ]
                    for it in range(KT):
                        for sn in range(NS):
                            nc.tensor.matmul(
                                pso[sn][:],
                                t_sb[it][:, c0 : c0 + 128],
                                e_sb[it][:, sn * 512 : (sn + 1) * 512],
                                start=(it == 0), stop=(it == KT - 1),
                            )
                    for sn in range(NS):
                        osb = o_pool.tile([128, 512], F32, name="osb", tag="osb")
                        if cnt % 2 == 0:
                            nc.scalar.copy(osb[:], pso[sn][:])
                        else:
                            nc.vector.tensor_copy(osb[:], pso[sn][:])
                        eng = nc.sync if cnt % 2 == 0 else nc.scalar
                        eng.dma_start(
                            outT[mc * 128 : (mc + 1) * 128, sn * 512 : (sn + 1) * 512],
                            osb[:],
                        )
                        cnt += 1

    nc.compile()
    return nc


def _prepare_a_in_maps(vulns):
    vulns = np.ascontiguousarray(np.asarray(vulns, dtype=np.float32))
    pair = np.ascontiguousarray(
        np.repeat(np.eye(ROWS_PER_CORE, dtype=np.float32), 2, axis=0)
    )
    in_maps = []
    for c in range(NCORES):
        vsh = vulns[c * ROWS_PER_CORE : (c + 1) * ROWS_PER_CORE]
        in_maps.append(
            {
                "v128": np.ascontiguousarray(vsh.reshape(128, NVT, VFREE)),
                "pairmat": pair,
            }
        )
    return in_maps


def _prepare_b_in_maps(embed_table, domain_ids, p_full, msum_full):
    embed_table = np.ascontiguousarray(np.asarray(embed_table, dtype=np.float32))
    domain_ids = np.asarray(domain_ids).astype(np.int64)
    E = np.ascontiguousarray(embed_table[domain_ids])  # [512, 2048]
    e4 = _tf32_round(E).reshape(KT, 128, S)
    # per-partition layout [128, 8]
    pm_pp = np.empty((128, 2 * KT), dtype=np.float32)
    pm_pp[:, 0:KT] = p_full.reshape(KT, 128).T
    pm_pp[:, KT : 2 * KT] = msum_full.reshape(KT, 128).T
    p_row = np.ascontiguousarray(p_full.astype(np.float32).reshape(1, D))
    ms_row = np.ascontiguousarray(msum_full.astype(np.float32).reshape(1, D))
    in_maps = []
    for c in range(NCORES):
        in_maps.append(
            {
                "pm_pp": pm_pp,
                "p_row": p_row,
                "ms_row": ms_row,
                "efull": e4,
                "ecols": np.ascontiguousarray(
                    e4[:, :, c * COLS_PER_CORE : (c + 1) * COLS_PER_CORE]
                ),
            }
        )
    return in_maps


def kernel(vulns, embed_table, domain_ids, _trace=False):
    if "nc_a" not in _CACHE:
        _CACHE["nc_a"] = build_kernel_a()
    if "nc_b" not in _CACHE:
        _CACHE["nc_b"] = build_kernel_b()

    res_a = run_bass_kernel_spmd(
        _CACHE["nc_a"], _prepare_a_in_maps(vulns),
        core_ids=list(range(NCORES)), trace=_trace,
    )
    _CACHE["res_a"] = res_a
    p_full = np.concatenate([res_a.results[c]["out_pm"][:, 0] for c in range(NCORES)])
    msum_full = np.concatenate(
        [res_a.results[c]["out_pm"][:, 1] for c in range(NCORES)]
    )

    res_b = run_bass_kernel_spmd(
        _CACHE["nc_b"], _prepare_b_in_maps(embed_table, domain_ids, p_full, msum_full),
        core_ids=list(range(NCORES)), trace=_trace,
    )
    _CACHE["res_b"] = res_b

    out = np.empty((S, S), dtype=np.complex64)
    for c in range(NCORES):
        r = res_b.results[c]
        sl = slice(c * COLS_PER_CORE, (c + 1) * COLS_PER_CORE)
        out[:, sl] = r["out_re"].T + 1j * r["out_im"].T
    return out


if __name__ == "__main__":
    rng = np.random.default_rng(0)
    v = rng.standard_normal((D, V), dtype=np.float32)
    et = rng.standard_normal((D, S), dtype=np.float32)
    ids = np.arange(D, dtype=np.int32)
    out = kernel(v, et, ids)
    print(out.shape, out.dtype)


# revision 15
# speedup vs baseline: 1.0815x; 1.0667x over previous
"""Trainium2 Bass kernel for nn_OmegaEntangle (E^T C E with entangle coefficients).

Math (validated vs reference to ~8e-7 rel err in fp32):
  p_i = sum_j v_ij^2 ; m_i = mean_j v_ij
  C[i,j] = mask(i<j) * sqrt(p_i p_j) * (m_i + 1j*m_j) / sqrt(m_i^2 + m_j^2)
  out = E^T C E   (complex, E real)  ->  out_re = E^T Cr E, out_im = E^T Ci E

Sharding: data-parallel over the 2048 OUTPUT COLUMNS (256 per core), with the
p/m reduction row-sharded (64 rows per core).

Two NEFF launches (a device collective would cost ~60+ us of entry-barrier +
AllGather latency on this platform for 768 bytes; host concat of the tiny
reduction result is far cheaper):
  Kernel A: each core reduces its [64, 32768] vuln shard -> p[64], msum[64].
  Host: concatenates the 8 shards (pure data movement, no math).
  Kernel B: each core derives sp/a/m2 vectors, builds C^T, computes
    T = C @ E[:, cols] and out[:, cols] = E^T @ T, writes [2048, 256] slabs.
Host concatenates slabs along columns -> [2048, 2048] complex64.
"""

import numpy as np

import concourse.bass as bass
import concourse.mybir as mybir
import concourse.tile as tile
from concourse import bacc
from concourse.bass_utils import run_bass_kernel_spmd

D = 512          # number of domains
V = 32768        # vuln dim
S = 2048         # sup (embed) dim
NCORES = 8
ROWS_PER_CORE = D // NCORES          # 64
COLS_PER_CORE = S // NCORES          # 256
NVT = 8                               # number of vuln tiles per core
VFREE = (ROWS_PER_CORE * V) // (128 * NVT)   # 2048 free elems per vuln tile
KT = D // 128                         # 4 contraction tiles
MT = S // 128                         # 16 output row tiles
INV_V = 1.0 / V
WARMUP_MMS = 16                       # PE warm-up matmuls at kernel-B start

F32 = mybir.dt.float32
F32R = mybir.dt.float32r
BF16 = mybir.dt.bfloat16
# float32r (TF32) matmul inputs stream at 1 cyc/row vs 4 for float32.
# Host pre-rounds E to TF32 values; on-device producers of matmul operands
# write float32r-typed tiles so the BIR verifier sees rounded inputs.


def _tf32_round(x):
    xi = np.ascontiguousarray(x, dtype=np.float32).view(np.uint32)
    return ((xi + np.uint32(0x1000)) & np.uint32(0xFFFFE000)).view(np.float32)
AF = mybir.ActivationFunctionType
ALU = mybir.AluOpType

_CACHE = {}


def build_kernel_a():
    """Reduce kernel: per-core p/msum over the 64-row vuln shard."""
    nc = bacc.Bacc("TRN2", target_bir_lowering=False, debug=False, num_devices=NCORES)

    v128 = nc.dram_tensor("v128", [128, NVT * VFREE], F32, kind="ExternalInput")
    pairmat = nc.dram_tensor("pairmat", [128, ROWS_PER_CORE], F32, kind="ExternalInput")
    out_pm = nc.dram_tensor("out_pm", [ROWS_PER_CORE, 2], F32, kind="ExternalOutput")
    widths = [2048] * 6 + [1024] * 4

    with tile.TileContext(nc) as tc:
        with (
            tc.tile_pool(name="vin", bufs=3) as vin_pool,
            tc.tile_pool(name="scr", bufs=2) as scr_pool,
            tc.tile_pool(name="small", bufs=1) as small_pool,
            tc.tile_pool(name="ps", bufs=1, space="PSUM") as ps_pool,
        ):
            vts = []
            off = 0
            for t, w in enumerate(widths):
                vt = vin_pool.tile([128, VFREE], F32, name=f"vt{t}", tag="vt")
                nc.sync.dma_start(vt[:, 0:w], v128[:, off : off + w])
                off += w
                vts.append(vt)
            pair_sb = small_pool.tile([128, ROWS_PER_CORE], F32, name="pair_sb")
            nc.sync.dma_start(pair_sb[:], pairmat[:])

            NT = len(widths)
            pm_acc = small_pool.tile([128, 2 * NT], F32, name="pm_acc")
            for t, w in enumerate(widths):
                sq = scr_pool.tile([128, VFREE], F32, name="sq", tag="sq")
                nc.scalar.activation(
                    sq[:, 0:w], vts[t][:, 0:w], AF.Square,
                    accum_out=pm_acc[:, t : t + 1],
                )
                raw = scr_pool.tile([128, VFREE], F32, name="raw", tag="raw")
                nc.vector.tensor_scalar(
                    raw[:, 0:w], vts[t][:, 0:w], 1.0, None, ALU.mult, ALU.add,
                    accum_out=pm_acc[:, NT + t : NT + t + 1],
                )

            ps_pm = ps_pool.tile([ROWS_PER_CORE, 2 * NT], F32, name="ps_pm")
            nc.tensor.matmul(ps_pm[:], pair_sb[:], pm_acc[:], start=True, stop=True)

            d2 = small_pool.tile([ROWS_PER_CORE, 2], F32, name="d2")
            nc.vector.tensor_reduce(
                d2[:, 0:1], ps_pm[:, 0:NT], mybir.AxisListType.X, ALU.add
            )
            nc.vector.tensor_reduce(
                d2[:, 1:2], ps_pm[:, NT : 2 * NT], mybir.AxisListType.X, ALU.add
            )
            nc.sync.dma_start(out_pm[:], d2[:])

    nc.compile()
    return nc


def build_kernel_b():
    """Main kernel: derive vectors, build C^T, two matmul chains, write slab."""
    nc = bacc.Bacc("TRN2", target_bir_lowering=False, debug=False, num_devices=NCORES)

    # pm_pp: per-partition layout, col kt   = p[q + 128*kt],
    #        col 4+kt = msum[q + 128*kt]    (q = partition)
    pm_pp = nc.dram_tensor("pm_pp", [128, 2 * KT], F32, kind="ExternalInput")
    # raw reduction outputs replicated across partitions (host-side replication)
    p_bc_in = nc.dram_tensor("p_bc", [128, D], F32, kind="ExternalInput")
    ms_bc_in = nc.dram_tensor("ms_bc", [128, D], F32, kind="ExternalInput")
    efull = nc.dram_tensor("efull", [KT, 128, S], F32R, kind="ExternalInput")
    ecols = nc.dram_tensor("ecols", [KT, 128, COLS_PER_CORE], F32R, kind="ExternalInput")
    # transposed output slabs: host transposes back (out[:, cols] = slab.T)
    out_re = nc.dram_tensor("out_re", [COLS_PER_CORE, S], F32, kind="ExternalOutput")
    out_im = nc.dram_tensor("out_im", [COLS_PER_CORE, S], F32, kind="ExternalOutput")

    with tile.TileContext(nc) as tc:
        with (
            tc.tile_pool(name="epool", bufs=1) as e_pool,
            tc.tile_pool(name="small", bufs=1) as small_pool,
            tc.tile_pool(name="cbuild", bufs=2) as cb_pool,
            tc.tile_pool(name="ctp", bufs=1) as ct_pool,
            tc.tile_pool(name="tsb", bufs=1) as t_pool,
            tc.tile_pool(name="ost", bufs=4) as o_pool,
            tc.tile_pool(name="psA", bufs=4, space="PSUM") as psA,
            tc.tile_pool(name="psB", bufs=4, space="PSUM") as psB,
        ):
            # -------- input DMAs (small first, then E) ------------------------
            pp = small_pool.tile([128, 2 * KT], F32, name="pp")
            nc.sync.dma_start(pp[:], pm_pp[:])
            p_bct = small_pool.tile([128, D], F32, name="p_bct")
            nc.sync.dma_start(p_bct[:], p_bc_in[:])
            ms_bct = small_pool.tile([128, D], F32, name="ms_bct")
            nc.sync.dma_start(ms_bct[:], ms_bc_in[:])

            ec_sb = []
            for kt in range(KT):
                ect = e_pool.tile(
                    [128, COLS_PER_CORE], F32R, name=f"ec{kt}", tag=f"ec{kt}"
                )
                nc.sync.dma_start(ect[:], ecols[kt])
                ec_sb.append(ect)
            e_sb = []
            for kt in range(KT):
                et = e_pool.tile([128, S], F32R, name=f"e{kt}", tag=f"e{kt}")
                nc.sync.dma_start(et[:], efull[kt])
                e_sb.append(et)

            # -------- PE warm-up during the small-vector derivation -----------
            warm_b = small_pool.tile([128, 512], BF16, name="warm_b")
            nc.gpsimd.memset(warm_b[:], 0.001)
            ps_w = psB.tile([128, 512], F32, name="ps_w", tag="o")
            for i in range(WARMUP_MMS):
                nc.tensor.matmul(
                    ps_w[:], warm_b[:, 0:128], warm_b[:],
                    start=(i == 0), stop=(i == WARMUP_MMS - 1),
                )

            # -------- derived vectors -----------------------------------------
            # per-partition [128, 4] each
            sp4 = small_pool.tile([128, KT], F32, name="sp4")
            a4 = small_pool.tile([128, KT], F32, name="a4")
            m24 = small_pool.tile([128, KT], F32, name="m24")
            nc.scalar.activation(sp4[:], pp[:, 0:KT], AF.Sqrt)
            nc.vector.scalar_tensor_tensor(
                a4[:], pp[:, KT : 2 * KT], INV_V, sp4[:], op0=ALU.mult, op1=ALU.mult
            )
            nc.vector.scalar_tensor_tensor(
                m24[:], pp[:, KT : 2 * KT], INV_V * INV_V, pp[:, KT : 2 * KT],
                op0=ALU.mult, op1=ALU.mult,
            )
            # broadcast derived tiles straight from the replicated raw inputs
            sp_bc = small_pool.tile([128, D], F32, name="sp_bc")
            a_bc = small_pool.tile([128, D], F32, name="a_bc")
            m2_bc = small_pool.tile([128, D], F32, name="m2_bc")
            nc.scalar.activation(sp_bc[:], p_bct[:], AF.Sqrt)
            nc.vector.scalar_tensor_tensor(
                a_bc[:], ms_bct[:], INV_V, sp_bc[:], op0=ALU.mult, op1=ALU.mult
            )
            nc.vector.scalar_tensor_tensor(
                m2_bc[:], ms_bct[:], INV_V * INV_V, ms_bct[:],
                op0=ALU.mult, op1=ALU.mult,
            )

            # -------- build C^T (real and imag) -------------------------------
            ct_r, ct_i = [], []
            for jt in range(KT):
                h = cb_pool.tile([128, D], F32, name="h", tag="h")
                nc.scalar.activation(
                    h[:], m2_bc[:], AF.Sqrt, bias=m24[:, jt : jt + 1], scale=1.0
                )
                rinv = cb_pool.tile([128, D], F32, name="rinv", tag="rinv")
                nc.vector.reciprocal_approx_fast(out=rinv[:], in_=h[:])
                rm = cb_pool.tile([128, D], F32, name="rm", tag="rm")
                nc.gpsimd.affine_select(
                    out=rm[:], in_=rinv[:],
                    pattern=[[-1, D]], compare_op=ALU.is_gt,
                    fill=0.0, base=128 * jt, channel_multiplier=1,
                )
                ctr = ct_pool.tile([128, D], F32R, name=f"ctr{jt}", tag=f"ctr{jt}")
                cti = ct_pool.tile([128, D], F32R, name=f"cti{jt}", tag=f"cti{jt}")
                nc.vector.scalar_tensor_tensor(
                    ctr[:], a_bc[:], sp4[:, jt : jt + 1], rm[:],
                    op0=ALU.mult, op1=ALU.mult,
                )
                nc.vector.scalar_tensor_tensor(
                    cti[:], sp_bc[:], a4[:, jt : jt + 1], rm[:],
                    op0=ALU.mult, op1=ALU.mult,
                )
                ct_r.append(ctr)
                ct_i.append(cti)

            # -------- T = C @ E[:, cols]  ([128, 512] = [T_r | T_i]) ----------
            ps_ts = [
                psA.tile(
                    [128, 2 * COLS_PER_CORE], F32, name=f"ps_t{it}", tag=f"t{it}",
                    bufs=1,
                )
                for it in range(KT)
            ]
            for part, cts in ((0, ct_r), (1, ct_i)):
                lo = part * COLS_PER_CORE
                for jt in range(KT):
                    for it in range(KT):
                        nc.tensor.matmul(
                            ps_ts[it][:, lo : lo + COLS_PER_CORE],
                            cts[jt][:, it * 128 : (it + 1) * 128],
                            ec_sb[jt][:],
                            start=(jt == 0), stop=(jt == KT - 1),
                        )
            t_sb = []
            for it in range(KT):
                tsb = t_pool.tile(
                    [128, 2 * COLS_PER_CORE], F32R, name=f"tsb{it}", tag=f"tsb{it}"
                )
                if it % 2 == 0:
                    nc.scalar.copy(tsb[:], ps_ts[it][:])
                else:
                    nc.vector.tensor_copy(tsb[:], ps_ts[it][:])
                t_sb.append(tsb)

            # -------- out^T[cols, :] = T^T @ E  (transposed chain) ------------
            # lhsT = T[i, c] slices straight from t_sb; rhs = e_sb 512-chunks.
            # Consecutive sn-matmuls share the same stationary operand.
            NS = S // 512
            cnt = 0
            for part, outT in ((0, out_re), (1, out_im)):
                for mc in range(2):
                    c0 = part * COLS_PER_CORE + mc * 128
                    pso = [
                        psB.tile([128, 512], F32, name=f"pso{sn}", tag="o")
                        for sn in range(NS)
                    ]
                    for it in range(KT):
                        for sn in range(NS):
                            nc.tensor.matmul(
                                pso[sn][:],
                                t_sb[it][:, c0 : c0 + 128],
                                e_sb[it][:, sn * 512 : (sn + 1) * 512],
                                start=(it == 0), stop=(it == KT - 1),
                            )
                    for sn in range(NS):
                        osb = o_pool.tile([128, 512], F32, name="osb", tag="osb")
                        if cnt % 2 == 0:
                            nc.scalar.copy(osb[:], pso[sn][:])
                        else:
                            nc.vector.tensor_copy(osb[:], pso[sn][:])
                        eng = nc.sync if cnt % 2 == 0 else nc.scalar
                        eng.dma_start(
                            outT[mc * 128 : (mc + 1) * 128, sn * 512 : (sn + 1) * 512],
                            osb[:],
                        )
                        cnt += 1

    nc.compile()
    return nc


def _prepare_a_in_maps(vulns):
    vulns = np.ascontiguousarray(np.asarray(vulns, dtype=np.float32))
    pair = np.ascontiguousarray(
        np.repeat(np.eye(ROWS_PER_CORE, dtype=np.float32), 2, axis=0)
    )
    in_maps = []
    for c in range(NCORES):
        vsh = vulns[c * ROWS_PER_CORE : (c + 1) * ROWS_PER_CORE]
        in_maps.append(
            {
                "v128": np.ascontiguousarray(vsh.reshape(128, NVT * VFREE)),
                "pairmat": pair,
            }
        )
    return in_maps


def _prepare_b_in_maps(embed_table, domain_ids, p_full, msum_full):
    embed_table = np.ascontiguousarray(np.asarray(embed_table, dtype=np.float32))
    domain_ids = np.asarray(domain_ids).astype(np.int64)
    E = np.ascontiguousarray(embed_table[domain_ids])  # [512, 2048]
    e4 = _tf32_round(E).reshape(KT, 128, S)
    # per-partition layout [128, 8]
    pm_pp = np.empty((128, 2 * KT), dtype=np.float32)
    pm_pp[:, 0:KT] = p_full.reshape(KT, 128).T
    pm_pp[:, KT : 2 * KT] = msum_full.reshape(KT, 128).T
    p_bc = np.ascontiguousarray(
        np.broadcast_to(p_full.astype(np.float32), (128, D))
    )
    ms_bc = np.ascontiguousarray(
        np.broadcast_to(msum_full.astype(np.float32), (128, D))
    )
    in_maps = []
    for c in range(NCORES):
        in_maps.append(
            {
                "pm_pp": pm_pp,
                "p_bc": p_bc,
                "ms_bc": ms_bc,
                "efull": e4,
                "ecols": np.ascontiguousarray(
                    e4[:, :, c * COLS_PER_CORE : (c + 1) * COLS_PER_CORE]
                ),
            }
        )
    return in_maps


def kernel(vulns, embed_table, domain_ids, _trace=False):
    if "nc_a" not in _CACHE:
        _CACHE["nc_a"] = build_kernel_a()
    if "nc_b" not in _CACHE:
        _CACHE["nc_b"] = build_kernel_b()

    res_a = run_bass_kernel_spmd(
        _CACHE["nc_a"], _prepare_a_in_maps(vulns),
        core_ids=list(range(NCORES)), trace=_trace,
    )
    _CACHE["res_a"] = res_a
    p_full = np.concatenate([res_a.results[c]["out_pm"][:, 0] for c in range(NCORES)])
    msum_full = np.concatenate(
        [res_a.results[c]["out_pm"][:, 1] for c in range(NCORES)]
    )

    res_b = run_bass_kernel_spmd(
        _CACHE["nc_b"], _prepare_b_in_maps(embed_table, domain_ids, p_full, msum_full),
        core_ids=list(range(NCORES)), trace=_trace,
    )
    _CACHE["res_b"] = res_b

    out = np.empty((S, S), dtype=np.complex64)
    for c in range(NCORES):
        r = res_b.results[c]
        sl = slice(c * COLS_PER_CORE, (c + 1) * COLS_PER_CORE)
        out[:, sl] = r["out_re"].T + 1j * r["out_im"].T
    return out


if __name__ == "__main__":
    rng = np.random.default_rng(0)
    v = rng.standard_normal((D, V), dtype=np.float32)
    et = rng.standard_normal((D, S), dtype=np.float32)
    ids = np.arange(D, dtype=np.int32)
    out = kernel(v, et, ids)
    print(out.shape, out.dtype)


# revision 16
# speedup vs baseline: 1.0883x; 1.0063x over previous
"""Trainium2 Bass kernel for nn_OmegaEntangle (E^T C E with entangle coefficients).

Math (validated vs reference to ~8e-7 rel err in fp32):
  p_i = sum_j v_ij^2 ; m_i = mean_j v_ij
  C[i,j] = mask(i<j) * sqrt(p_i p_j) * (m_i + 1j*m_j) / sqrt(m_i^2 + m_j^2)
  out = E^T C E   (complex, E real)  ->  out_re = E^T Cr E, out_im = E^T Ci E

Sharding: data-parallel over the 2048 OUTPUT COLUMNS (256 per core), with the
p/m reduction row-sharded (64 rows per core).

Two NEFF launches (a device collective would cost ~60+ us of entry-barrier +
AllGather latency on this platform for 768 bytes; host concat of the tiny
reduction result is far cheaper):
  Kernel A: each core reduces its [64, 32768] vuln shard -> p[64], msum[64].
  Host: concatenates the 8 shards (pure data movement, no math).
  Kernel B: each core derives sp/a/m2 vectors, builds C^T, computes
    T = C @ E[:, cols] and out[:, cols] = E^T @ T, writes [2048, 256] slabs.
Host concatenates slabs along columns -> [2048, 2048] complex64.
"""

import numpy as np

import concourse.bass as bass
import concourse.mybir as mybir
import concourse.tile as tile
from concourse import bacc
from concourse.bass_utils import run_bass_kernel_spmd

D = 512          # number of domains
V = 32768        # vuln dim
S = 2048         # sup (embed) dim
NCORES = 8
ROWS_PER_CORE = D // NCORES          # 64
COLS_PER_CORE = S // NCORES          # 256
NVT = 8                               # number of vuln tiles per core
VFREE = (ROWS_PER_CORE * V) // (128 * NVT)   # 2048 free elems per vuln tile
KT = D // 128                         # 4 contraction tiles
MT = S // 128                         # 16 output row tiles
INV_V = 1.0 / V
WARMUP_MMS = 22                       # PE warm-up matmuls at kernel-B start

F32 = mybir.dt.float32
F32R = mybir.dt.float32r
BF16 = mybir.dt.bfloat16
# float32r (TF32) matmul inputs stream at 1 cyc/row vs 4 for float32.
# Host pre-rounds E to TF32 values; on-device producers of matmul operands
# write float32r-typed tiles so the BIR verifier sees rounded inputs.


def _tf32_round(x):
    xi = np.ascontiguousarray(x, dtype=np.float32).view(np.uint32)
    return ((xi + np.uint32(0x1000)) & np.uint32(0xFFFFE000)).view(np.float32)
AF = mybir.ActivationFunctionType
ALU = mybir.AluOpType

_CACHE = {}


def build_kernel_a():
    """Reduce kernel: per-core p/msum over the 64-row vuln shard."""
    nc = bacc.Bacc("TRN2", target_bir_lowering=False, debug=False, num_devices=NCORES)

    v128 = nc.dram_tensor("v128", [128, NVT * VFREE], F32, kind="ExternalInput")
    pairmat = nc.dram_tensor("pairmat", [128, ROWS_PER_CORE], F32, kind="ExternalInput")
    out_pm = nc.dram_tensor("out_pm", [ROWS_PER_CORE, 2], F32, kind="ExternalOutput")
    widths = [2048] * 6 + [1024] * 4

    with tile.TileContext(nc) as tc:
        with (
            tc.tile_pool(name="vin", bufs=3) as vin_pool,
            tc.tile_pool(name="scr", bufs=2) as scr_pool,
            tc.tile_pool(name="small", bufs=1) as small_pool,
            tc.tile_pool(name="ps", bufs=1, space="PSUM") as ps_pool,
        ):
            vts = []
            off = 0
            for t, w in enumerate(widths):
                vt = vin_pool.tile([128, VFREE], F32, name=f"vt{t}", tag="vt")
                nc.sync.dma_start(vt[:, 0:w], v128[:, off : off + w])
                off += w
                vts.append(vt)
            pair_sb = small_pool.tile([128, ROWS_PER_CORE], F32, name="pair_sb")
            nc.sync.dma_start(pair_sb[:], pairmat[:])

            NT = len(widths)
            pm_acc = small_pool.tile([128, 2 * NT], F32, name="pm_acc")
            for t, w in enumerate(widths):
                sq = scr_pool.tile([128, VFREE], F32, name="sq", tag="sq")
                nc.scalar.activation(
                    sq[:, 0:w], vts[t][:, 0:w], AF.Square,
                    accum_out=pm_acc[:, t : t + 1],
                )
                raw = scr_pool.tile([128, VFREE], F32, name="raw", tag="raw")
                nc.vector.tensor_scalar(
                    raw[:, 0:w], vts[t][:, 0:w], 1.0, None, ALU.mult, ALU.add,
                    accum_out=pm_acc[:, NT + t : NT + t + 1],
                )

            ps_pm = ps_pool.tile([ROWS_PER_CORE, 2 * NT], F32, name="ps_pm")
            nc.tensor.matmul(ps_pm[:], pair_sb[:], pm_acc[:], start=True, stop=True)

            d2 = small_pool.tile([ROWS_PER_CORE, 2], F32, name="d2")
            nc.vector.tensor_reduce(
                d2[:, 0:1], ps_pm[:, 0:NT], mybir.AxisListType.X, ALU.add
            )
            nc.vector.tensor_reduce(
                d2[:, 1:2], ps_pm[:, NT : 2 * NT], mybir.AxisListType.X, ALU.add
            )
            nc.sync.dma_start(out_pm[:], d2[:])

    nc.compile()
    return nc


def build_kernel_b():
    """Main kernel: derive vectors, build C^T, two matmul chains, write slab."""
    nc = bacc.Bacc("TRN2", target_bir_lowering=False, debug=False, num_devices=NCORES)

    # pm_pp: per-partition layout, col kt   = p[q + 128*kt],
    #        col 4+kt = msum[q + 128*kt]    (q = partition)
    pm_pp = nc.dram_tensor("pm_pp", [128, 2 * KT], F32, kind="ExternalInput")
    # raw reduction outputs replicated across partitions (host-side replication)
    p_bc_in = nc.dram_tensor("p_bc", [128, D], F32, kind="ExternalInput")
    ms_bc_in = nc.dram_tensor("ms_bc", [128, D], F32, kind="ExternalInput")
    efull = nc.dram_tensor("efull", [KT, 128, S], F32R, kind="ExternalInput")
    ecols = nc.dram_tensor("ecols", [KT, 128, COLS_PER_CORE], F32R, kind="ExternalInput")
    # transposed output slabs: host transposes back (out[:, cols] = slab.T)
    out_re = nc.dram_tensor("out_re", [COLS_PER_CORE, S], F32, kind="ExternalOutput")
    out_im = nc.dram_tensor("out_im", [COLS_PER_CORE, S], F32, kind="ExternalOutput")

    with tile.TileContext(nc) as tc:
        with (
            tc.tile_pool(name="epool", bufs=1) as e_pool,
            tc.tile_pool(name="small", bufs=1) as small_pool,
            tc.tile_pool(name="cbuild", bufs=2) as cb_pool,
            tc.tile_pool(name="ctp", bufs=1) as ct_pool,
            tc.tile_pool(name="tsb", bufs=1) as t_pool,
            tc.tile_pool(name="ost", bufs=4) as o_pool,
            tc.tile_pool(name="psA", bufs=4, space="PSUM") as psA,
            tc.tile_pool(name="psB", bufs=4, space="PSUM") as psB,
        ):
            # -------- input DMAs (small first, then E) ------------------------
            pp = small_pool.tile([128, 2 * KT], F32, name="pp")
            nc.sync.dma_start(pp[:], pm_pp[:])
            p_bct = small_pool.tile([128, D], F32, name="p_bct")
            nc.sync.dma_start(p_bct[:], p_bc_in[:])
            ms_bct = small_pool.tile([128, D], F32, name="ms_bct")
            nc.sync.dma_start(ms_bct[:], ms_bc_in[:])

            ec_sb = []
            for kt in range(KT):
                ect = e_pool.tile(
                    [128, COLS_PER_CORE], F32R, name=f"ec{kt}", tag=f"ec{kt}"
                )
                nc.sync.dma_start(ect[:], ecols[kt])
                ec_sb.append(ect)
            e_sb = []
            for kt in range(KT):
                et = e_pool.tile([128, S], F32R, name=f"e{kt}", tag=f"e{kt}")
                nc.sync.dma_start(et[:], efull[kt])
                e_sb.append(et)

            # -------- PE warm-up during the small-vector derivation -----------
            warm_b = small_pool.tile([128, 512], BF16, name="warm_b")
            nc.gpsimd.memset(warm_b[:], 0.001)
            ps_w = psB.tile([128, 512], F32, name="ps_w", tag="o")
            for i in range(WARMUP_MMS):
                nc.tensor.matmul(
                    ps_w[:], warm_b[:, 0:128], warm_b[:],
                    start=(i == 0), stop=(i == WARMUP_MMS - 1),
                )

            # -------- derived vectors -----------------------------------------
            # per-partition [128, 4] each
            sp4 = small_pool.tile([128, KT], F32, name="sp4")
            a4 = small_pool.tile([128, KT], F32, name="a4")
            m24 = small_pool.tile([128, KT], F32, name="m24")
            nc.vector.scalar_tensor_tensor(
                m24[:], pp[:, KT : 2 * KT], INV_V * INV_V, pp[:, KT : 2 * KT],
                op0=ALU.mult, op1=ALU.mult,
            )
            nc.scalar.activation(sp4[:], pp[:, 0:KT], AF.Sqrt)
            nc.vector.scalar_tensor_tensor(
                a4[:], pp[:, KT : 2 * KT], INV_V, sp4[:], op0=ALU.mult, op1=ALU.mult
            )
            # broadcast derived tiles straight from the replicated raw inputs
            sp_bc = small_pool.tile([128, D], F32, name="sp_bc")
            a_bc = small_pool.tile([128, D], F32, name="a_bc")
            m2_bc = small_pool.tile([128, D], F32, name="m2_bc")
            nc.vector.scalar_tensor_tensor(
                m2_bc[:], ms_bct[:], INV_V * INV_V, ms_bct[:],
                op0=ALU.mult, op1=ALU.mult,
            )
            nc.scalar.activation(sp_bc[:], p_bct[:], AF.Sqrt)
            nc.vector.scalar_tensor_tensor(
                a_bc[:], ms_bct[:], INV_V, sp_bc[:], op0=ALU.mult, op1=ALU.mult
            )

            # -------- build C^T (real and imag) -------------------------------
            ct_r, ct_i = [], []
            for jt in range(KT):
                h = cb_pool.tile([128, D], F32, name="h", tag="h")
                nc.scalar.activation(
                    h[:], m2_bc[:], AF.Sqrt, bias=m24[:, jt : jt + 1], scale=1.0
                )
                rinv = cb_pool.tile([128, D], F32, name="rinv", tag="rinv")
                nc.vector.reciprocal_approx_fast(out=rinv[:], in_=h[:])
                rm = cb_pool.tile([128, D], F32, name="rm", tag="rm")
                nc.gpsimd.affine_select(
                    out=rm[:], in_=rinv[:],
                    pattern=[[-1, D]], compare_op=ALU.is_gt,
                    fill=0.0, base=128 * jt, channel_multiplier=1,
                )
                ctr = ct_pool.tile([128, D], F32R, name=f"ctr{jt}", tag=f"ctr{jt}")
                cti = ct_pool.tile([128, D], F32R, name=f"cti{jt}", tag=f"cti{jt}")
                nc.vector.scalar_tensor_tensor(
                    ctr[:], a_bc[:], sp4[:, jt : jt + 1], rm[:],
                    op0=ALU.mult, op1=ALU.mult,
                )
                nc.vector.scalar_tensor_tensor(
                    cti[:], sp_bc[:], a4[:, jt : jt + 1], rm[:],
                    op0=ALU.mult, op1=ALU.mult,
                )
                ct_r.append(ctr)
                ct_i.append(cti)

            # -------- T = C @ E[:, cols]  ([128, 512] = [T_r | T_i]) ----------
            ps_ts = [
                psA.tile(
                    [128, 2 * COLS_PER_CORE], F32, name=f"ps_t{it}", tag=f"t{it}",
                    bufs=1,
                )
                for it in range(KT)
            ]
            for part, cts in ((0, ct_r), (1, ct_i)):
                lo = part * COLS_PER_CORE
                for jt in range(KT):
                    for it in range(KT):
                        nc.tensor.matmul(
                            ps_ts[it][:, lo : lo + COLS_PER_CORE],
                            cts[jt][:, it * 128 : (it + 1) * 128],
                            ec_sb[jt][:],
                            start=(jt == 0), stop=(jt == KT - 1),
                        )
            t_sb = []
            for it in range(KT):
                tsb = t_pool.tile(
                    [128, 2 * COLS_PER_CORE], F32R, name=f"tsb{it}", tag=f"tsb{it}"
                )
                if it % 2 == 0:
                    nc.scalar.copy(tsb[:], ps_ts[it][:])
                else:
                    nc.vector.tensor_copy(tsb[:], ps_ts[it][:])
                t_sb.append(tsb)

            # -------- out^T[cols, :] = T^T @ E  (transposed chain) ------------
            # lhsT = T[i, c] slices straight from t_sb; rhs = e_sb 512-chunks.
            # Consecutive sn-matmuls share the same stationary operand.
            NS = S // 512
            cnt = 0
            for part, outT in ((0, out_re), (1, out_im)):
                for mc in range(2):
                    c0 = part * COLS_PER_CORE + mc * 128
                    pso = [
                        psB.tile([128, 512], F32, name=f"pso{sn}", tag="o")
                        for sn in range(NS)
                    ]
                    for it in range(KT):
                        for sn in range(NS):
                            nc.tensor.matmul(
                                pso[sn][:],
                                t_sb[it][:, c0 : c0 + 128],
                                e_sb[it][:, sn * 512 : (sn + 1) * 512],
                                start=(it == 0), stop=(it == KT - 1),
                            )
                    for sn in range(NS):
                        osb = o_pool.tile([128, 512], F32, name="osb", tag="osb")
                        if cnt % 2 == 0:
                            nc.scalar.copy(osb[:], pso[sn][:])
                        else:
                            nc.vector.tensor_copy(osb[:], pso[sn][:])
                        eng = nc.sync if cnt % 2 == 0 else nc.scalar
                        eng.dma_start(
                            outT[mc * 128 : (mc + 1) * 128, sn * 512 : (sn + 1) * 512],
                            osb[:],
                        )
                        cnt += 1

    nc.compile()
    return nc


def _prepare_a_in_maps(vulns):
    vulns = np.ascontiguousarray(np.asarray(vulns, dtype=np.float32))
    pair = np.ascontiguousarray(
        np.repeat(np.eye(ROWS_PER_CORE, dtype=np.float32), 2, axis=0)
    )
    in_maps = []
    for c in range(NCORES):
        vsh = vulns[c * ROWS_PER_CORE : (c + 1) * ROWS_PER_CORE]
        in_maps.append(
            {
                "v128": np.ascontiguousarray(vsh.reshape(128, NVT * VFREE)),
                "pairmat": pair,
            }
        )
    return in_maps


def _prepare_b_in_maps(embed_table, domain_ids, p_full, msum_full):
    embed_table = np.ascontiguousarray(np.asarray(embed_table, dtype=np.float32))
    domain_ids = np.asarray(domain_ids).astype(np.int64)
    E = np.ascontiguousarray(embed_table[domain_ids])  # [512, 2048]
    e4 = _tf32_round(E).reshape(KT, 128, S)
    # per-partition layout [128, 8]
    pm_pp = np.empty((128, 2 * KT), dtype=np.float32)
    pm_pp[:, 0:KT] = p_full.reshape(KT, 128).T
    pm_pp[:, KT : 2 * KT] = msum_full.reshape(KT, 128).T
    p_bc = np.ascontiguousarray(
        np.broadcast_to(p_full.astype(np.float32), (128, D))
    )
    ms_bc = np.ascontiguousarray(
        np.broadcast_to(msum_full.astype(np.float32), (128, D))
    )
    in_maps = []
    for c in range(NCORES):
        in_maps.append(
            {
                "pm_pp": pm_pp,
                "p_bc": p_bc,
                "ms_bc": ms_bc,
                "efull": e4,
                "ecols": np.ascontiguousarray(
                    e4[:, :, c * COLS_PER_CORE : (c + 1) * COLS_PER_CORE]
                ),
            }
        )
    return in_maps


def kernel(vulns, embed_table, domain_ids, _trace=False):
    if "nc_a" not in _CACHE:
        _CACHE["nc_a"] = build_kernel_a()
    if "nc_b" not in _CACHE:
        _CACHE["nc_b"] = build_kernel_b()

    res_a = run_bass_kernel_spmd(
        _CACHE["nc_a"], _prepare_a_in_maps(vulns),
        core_ids=list(range(NCORES)), trace=_trace,
    )
    _CACHE["res_a"] = res_a
    p_full = np.concatenate([res_a.results[c]["out_pm"][:, 0] for c in range(NCORES)])
    msum_full = np.concatenate(
        [res_a.results[c]["out_pm"][:, 1] for c in range(NCORES)]
    )

    res_b = run_bass_kernel_spmd(
        _CACHE["nc_b"], _prepare_b_in_maps(embed_table, domain_ids, p_full, msum_full),
        core_ids=list(range(NCORES)), trace=_trace,
    )
    _CACHE["res_b"] = res_b

    out = np.empty((S, S), dtype=np.complex64)
    for c in range(NCORES):
        r = res_b.results[c]
        sl = slice(c * COLS_PER_CORE, (c + 1) * COLS_PER_CORE)
        out[:, sl] = r["out_re"].T + 1j * r["out_im"].T
    return out


if __name__ == "__main__":
    rng = np.random.default_rng(0)
    v = rng.standard_normal((D, V), dtype=np.float32)
    et = rng.standard_normal((D, S), dtype=np.float32)
    ids = np.arange(D, dtype=np.int32)
    out = kernel(v, et, ids)
    print(out.shape, out.dtype)


# revision 18
# speedup vs baseline: 1.0927x; 1.0040x over previous
"""Trainium2 Bass kernel for nn_OmegaEntangle (E^T C E with entangle coefficients).

Math (validated vs reference to ~8e-7 rel err in fp32):
  p_i = sum_j v_ij^2 ; m_i = mean_j v_ij
  C[i,j] = mask(i<j) * sqrt(p_i p_j) * (m_i + 1j*m_j) / sqrt(m_i^2 + m_j^2)
  out = E^T C E   (complex, E real)  ->  out_re = E^T Cr E, out_im = E^T Ci E

Sharding: data-parallel over the 2048 OUTPUT COLUMNS (256 per core), with the
p/m reduction row-sharded (64 rows per core).

Two NEFF launches (a device collective would cost ~60+ us of entry-barrier +
AllGather latency on this platform for 768 bytes; host concat of the tiny
reduction result is far cheaper):
  Kernel A: each core reduces its [64, 32768] vuln shard -> p[64], msum[64].
  Host: concatenates the 8 shards (pure data movement, no math).
  Kernel B: each core derives sp/a/m2 vectors, builds C^T, computes
    T = C @ E[:, cols] and out[:, cols] = E^T @ T, writes [2048, 256] slabs.
Host concatenates slabs along columns -> [2048, 2048] complex64.
"""

import numpy as np

import concourse.bass as bass
import concourse.mybir as mybir
import concourse.tile as tile
from concourse import bacc
from concourse.bass_utils import run_bass_kernel_spmd

D = 512          # number of domains
V = 32768        # vuln dim
S = 2048         # sup (embed) dim
NCORES = 8
ROWS_PER_CORE = D // NCORES          # 64
COLS_PER_CORE = S // NCORES          # 256
NVT = 8                               # number of vuln tiles per core
VFREE = (ROWS_PER_CORE * V) // (128 * NVT)   # 2048 free elems per vuln tile
KT = D // 128                         # 4 contraction tiles
MT = S // 128                         # 16 output row tiles
INV_V = 1.0 / V
WARMUP_MMS = 30                       # PE warm-up matmuls at kernel-B start

F32 = mybir.dt.float32
F32R = mybir.dt.float32r
BF16 = mybir.dt.bfloat16
# float32r (TF32) matmul inputs stream at 1 cyc/row vs 4 for float32.
# Host pre-rounds E to TF32 values; on-device producers of matmul operands
# write float32r-typed tiles so the BIR verifier sees rounded inputs.


def _tf32_round(x):
    xi = np.ascontiguousarray(x, dtype=np.float32).view(np.uint32)
    return ((xi + np.uint32(0x1000)) & np.uint32(0xFFFFE000)).view(np.float32)
AF = mybir.ActivationFunctionType
ALU = mybir.AluOpType

_CACHE = {}


def build_kernel_a():
    """Reduce kernel: per-core p/msum over the 64-row vuln shard."""
    nc = bacc.Bacc("TRN2", target_bir_lowering=False, debug=False, num_devices=NCORES)

    v128 = nc.dram_tensor("v128", [128, NVT * VFREE], F32, kind="ExternalInput")
    pairmat = nc.dram_tensor("pairmat", [128, ROWS_PER_CORE], F32, kind="ExternalInput")
    out_pm = nc.dram_tensor("out_pm", [ROWS_PER_CORE, 2], F32, kind="ExternalOutput")
    widths = [2048] * 6 + [1024] * 4

    with tile.TileContext(nc) as tc:
        with (
            tc.tile_pool(name="vin", bufs=3) as vin_pool,
            tc.tile_pool(name="scr", bufs=2) as scr_pool,
            tc.tile_pool(name="small", bufs=1) as small_pool,
            tc.tile_pool(name="ps", bufs=1, space="PSUM") as ps_pool,
        ):
            vts = []
            off = 0
            for t, w in enumerate(widths):
                vt = vin_pool.tile([128, VFREE], F32, name=f"vt{t}", tag="vt")
                nc.sync.dma_start(vt[:, 0:w], v128[:, off : off + w])
                off += w
                vts.append(vt)
            pair_sb = small_pool.tile([128, ROWS_PER_CORE], F32, name="pair_sb")
            nc.sync.dma_start(pair_sb[:], pairmat[:])

            NT = len(widths)
            pm_acc = small_pool.tile([128, 2 * NT], F32, name="pm_acc")
            for t, w in enumerate(widths):
                sq = scr_pool.tile([128, VFREE], F32, name="sq", tag="sq")
                nc.scalar.activation(
                    sq[:, 0:w], vts[t][:, 0:w], AF.Square,
                    accum_out=pm_acc[:, t : t + 1],
                )
                raw = scr_pool.tile([128, VFREE], F32, name="raw", tag="raw")
                nc.vector.tensor_scalar(
                    raw[:, 0:w], vts[t][:, 0:w], 1.0, None, ALU.mult, ALU.add,
                    accum_out=pm_acc[:, NT + t : NT + t + 1],
                )

            ps_pm = ps_pool.tile([ROWS_PER_CORE, 2 * NT], F32, name="ps_pm")
            nc.tensor.matmul(ps_pm[:], pair_sb[:], pm_acc[:], start=True, stop=True)

            d2 = small_pool.tile([ROWS_PER_CORE, 2], F32, name="d2")
            nc.vector.tensor_reduce(
                d2[:, 0:1], ps_pm[:, 0:NT], mybir.AxisListType.X, ALU.add
            )
            nc.vector.tensor_reduce(
                d2[:, 1:2], ps_pm[:, NT : 2 * NT], mybir.AxisListType.X, ALU.add
            )
            nc.sync.dma_start(out_pm[:], d2[:])

    nc.compile()
    return nc


def build_kernel_b():
    """Main kernel: derive vectors, build C^T, two matmul chains, write slab."""
    nc = bacc.Bacc("TRN2", target_bir_lowering=False, debug=False, num_devices=NCORES)

    # pm_pp: per-partition layout, col kt   = p[q + 128*kt],
    #        col 4+kt = msum[q + 128*kt]    (q = partition)
    pm_pp = nc.dram_tensor("pm_pp", [128, 2 * KT], F32, kind="ExternalInput")
    # raw reduction outputs replicated across partitions (host-side replication)
    p_bc_in = nc.dram_tensor("p_bc", [128, D], F32, kind="ExternalInput")
    ms_bc_in = nc.dram_tensor("ms_bc", [128, D], F32, kind="ExternalInput")
    efull = nc.dram_tensor("efull", [KT, 128, S], F32R, kind="ExternalInput")
    ecols = nc.dram_tensor("ecols", [KT, 128, COLS_PER_CORE], F32R, kind="ExternalInput")
    # transposed output slabs: host transposes back (out[:, cols] = slab.T)
    out_re = nc.dram_tensor("out_re", [COLS_PER_CORE, S], F32, kind="ExternalOutput")
    out_im = nc.dram_tensor("out_im", [COLS_PER_CORE, S], F32, kind="ExternalOutput")

    with tile.TileContext(nc) as tc:
        with (
            tc.tile_pool(name="epool", bufs=1) as e_pool,
            tc.tile_pool(name="small", bufs=1) as small_pool,
            tc.tile_pool(name="cbuild", bufs=2) as cb_pool,
            tc.tile_pool(name="ctp", bufs=1) as ct_pool,
            tc.tile_pool(name="tsb", bufs=1) as t_pool,
            tc.tile_pool(name="ost", bufs=4) as o_pool,
            tc.tile_pool(name="psA", bufs=4, space="PSUM") as psA,
            tc.tile_pool(name="psB", bufs=4, space="PSUM") as psB,
        ):
            # -------- input DMAs (small first, then E) ------------------------
            pp = small_pool.tile([128, 2 * KT], F32, name="pp")
            nc.sync.dma_start(pp[:], pm_pp[:])
            p_bct = small_pool.tile([128, D], F32, name="p_bct")
            nc.sync.dma_start(p_bct[:], p_bc_in[:])
            ms_bct = small_pool.tile([128, D], F32, name="ms_bct")
            nc.sync.dma_start(ms_bct[:], ms_bc_in[:])

            ec_sb = []
            for kt in range(KT):
                ect = e_pool.tile(
                    [128, COLS_PER_CORE], F32R, name=f"ec{kt}", tag=f"ec{kt}"
                )
                nc.sync.dma_start(ect[:], ecols[kt])
                ec_sb.append(ect)
            e_sb = []
            for kt in range(KT):
                et = e_pool.tile([128, S], F32R, name=f"e{kt}", tag=f"e{kt}")
                nc.sync.dma_start(et[:], efull[kt])
                e_sb.append(et)

            # -------- PE warm-up during the small-vector derivation -----------
            warm_b = small_pool.tile([128, 512], BF16, name="warm_b")
            nc.gpsimd.memset(warm_b[:], 0.001)
            ps_w = psB.tile([128, 512], F32, name="ps_w", tag="o")
            for i in range(WARMUP_MMS):
                nc.tensor.matmul(
                    ps_w[:], warm_b[:, 0:128], warm_b[:],
                    start=(i == 0), stop=(i == WARMUP_MMS - 1),
                )

            # -------- derived vectors -----------------------------------------
            # per-partition [128, 4] each
            sp4 = small_pool.tile([128, KT], F32, name="sp4")
            a4 = small_pool.tile([128, KT], F32, name="a4")
            m24 = small_pool.tile([128, KT], F32, name="m24")
            nc.vector.scalar_tensor_tensor(
                m24[:], pp[:, KT : 2 * KT], INV_V * INV_V, pp[:, KT : 2 * KT],
                op0=ALU.mult, op1=ALU.mult,
            )
            nc.scalar.activation(sp4[:], pp[:, 0:KT], AF.Sqrt)
            nc.vector.scalar_tensor_tensor(
                a4[:], pp[:, KT : 2 * KT], INV_V, sp4[:], op0=ALU.mult, op1=ALU.mult
            )
            # broadcast derived tiles straight from the replicated raw inputs
            sp_bc = small_pool.tile([128, D], F32, name="sp_bc")
            a_bc = small_pool.tile([128, D], F32, name="a_bc")
            m2_bc = small_pool.tile([128, D], F32, name="m2_bc")
            nc.vector.scalar_tensor_tensor(
                m2_bc[:], ms_bct[:], INV_V * INV_V, ms_bct[:],
                op0=ALU.mult, op1=ALU.mult,
            )
            nc.scalar.activation(sp_bc[:], p_bct[:], AF.Sqrt)
            nc.vector.scalar_tensor_tensor(
                a_bc[:], ms_bct[:], INV_V, sp_bc[:], op0=ALU.mult, op1=ALU.mult
            )

            # -------- build C^T (real and imag) -------------------------------
            ct_r, ct_i = [], []
            for jt in range(KT):
                h = cb_pool.tile([128, D], F32, name="h", tag="h")
                nc.scalar.activation(
                    h[:], m2_bc[:], AF.Sqrt, bias=m24[:, jt : jt + 1], scale=1.0
                )
                rinv = cb_pool.tile([128, D], F32, name="rinv", tag="rinv")
                nc.vector.reciprocal_approx_fast(out=rinv[:], in_=h[:])
                rm = cb_pool.tile([128, D], F32, name="rm", tag="rm")
                nc.gpsimd.affine_select(
                    out=rm[:], in_=rinv[:],
                    pattern=[[-1, D]], compare_op=ALU.is_gt,
                    fill=0.0, base=128 * jt, channel_multiplier=1,
                )
                ctr = ct_pool.tile([128, D], F32R, name=f"ctr{jt}", tag=f"ctr{jt}")
                cti = ct_pool.tile([128, D], F32R, name=f"cti{jt}", tag=f"cti{jt}")
                nc.vector.scalar_tensor_tensor(
                    ctr[:], a_bc[:], sp4[:, jt : jt + 1], rm[:],
                    op0=ALU.mult, op1=ALU.mult,
                )
                nc.vector.scalar_tensor_tensor(
                    cti[:], sp_bc[:], a4[:, jt : jt + 1], rm[:],
                    op0=ALU.mult, op1=ALU.mult,
                )
                ct_r.append(ctr)
                ct_i.append(cti)

            # -------- T = C @ E[:, cols]  ([128, 512] = [T_r | T_i]) ----------
            ps_ts = [
                psA.tile(
                    [128, 2 * COLS_PER_CORE], F32, name=f"ps_t{it}", tag=f"t{it}",
                    bufs=1,
                )
                for it in range(KT)
            ]
            for part, cts in ((0, ct_r), (1, ct_i)):
                lo = part * COLS_PER_CORE
                for jt in range(KT):
                    for it in range(KT):
                        nc.tensor.matmul(
                            ps_ts[it][:, lo : lo + COLS_PER_CORE],
                            cts[jt][:, it * 128 : (it + 1) * 128],
                            ec_sb[jt][:],
                            start=(jt == 0), stop=(jt == KT - 1),
                        )
            t_sb = []
            for it in range(KT):
                tsb = t_pool.tile(
                    [128, 2 * COLS_PER_CORE], F32R, name=f"tsb{it}", tag=f"tsb{it}"
                )
                if it % 2 == 0:
                    nc.scalar.copy(tsb[:], ps_ts[it][:])
                else:
                    nc.vector.tensor_copy(tsb[:], ps_ts[it][:])
                t_sb.append(tsb)

            # -------- out^T[cols, :] = T^T @ E  (transposed chain) ------------
            # lhsT = T[i, c] slices straight from t_sb; rhs = e_sb 512-chunks.
            # Consecutive sn-matmuls share the same stationary operand.
            NS = S // 512
            cnt = 0
            for part, outT in ((0, out_re), (1, out_im)):
                for mc in range(2):
                    c0 = part * COLS_PER_CORE + mc * 128
                    pso = [
                        psB.tile([128, 512], F32, name=f"pso{sn}", tag="o")
                        for sn in range(NS)
                    ]
                    for it in range(KT):
                        for sn in range(NS):
                            nc.tensor.matmul(
                                pso[sn][:],
                                t_sb[it][:, c0 : c0 + 128],
                                e_sb[it][:, sn * 512 : (sn + 1) * 512],
                                start=(it == 0), stop=(it == KT - 1),
                            )
                    for sn in range(NS):
                        osb = o_pool.tile([128, 512], F32, name="osb", tag="osb")
                        if cnt % 2 == 0:
                            nc.scalar.copy(osb[:], pso[sn][:])
                        else:
                            nc.vector.tensor_copy(osb[:], pso[sn][:])
                        eng = nc.sync if cnt % 2 == 0 else nc.scalar
                        eng.dma_start(
                            outT[mc * 128 : (mc + 1) * 128, sn * 512 : (sn + 1) * 512],
                            osb[:],
                        )
                        cnt += 1

    nc.compile()
    return nc


def _prepare_a_in_maps(vulns):
    vulns = np.ascontiguousarray(np.asarray(vulns, dtype=np.float32))
    pair = np.ascontiguousarray(
        np.repeat(np.eye(ROWS_PER_CORE, dtype=np.float32), 2, axis=0)
    )
    in_maps = []
    for c in range(NCORES):
        vsh = vulns[c * ROWS_PER_CORE : (c + 1) * ROWS_PER_CORE]
        in_maps.append(
            {
                "v128": np.ascontiguousarray(vsh.reshape(128, NVT * VFREE)),
                "pairmat": pair,
            }
        )
    return in_maps


def _prepare_b_in_maps(embed_table, domain_ids, p_full, msum_full):
    embed_table = np.ascontiguousarray(np.asarray(embed_table, dtype=np.float32))
    domain_ids = np.asarray(domain_ids).astype(np.int64)
    E = np.ascontiguousarray(embed_table[domain_ids])  # [512, 2048]
    e4 = _tf32_round(E).reshape(KT, 128, S)
    # per-partition layout [128, 8]
    pm_pp = np.empty((128, 2 * KT), dtype=np.float32)
    pm_pp[:, 0:KT] = p_full.reshape(KT, 128).T
    pm_pp[:, KT : 2 * KT] = msum_full.reshape(KT, 128).T
    p_bc = np.ascontiguousarray(
        np.broadcast_to(p_full.astype(np.float32), (128, D))
    )
    ms_bc = np.ascontiguousarray(
        np.broadcast_to(msum_full.astype(np.float32), (128, D))
    )
    in_maps = []
    for c in range(NCORES):
        in_maps.append(
            {
                "pm_pp": pm_pp,
                "p_bc": p_bc,
                "ms_bc": ms_bc,
                "efull": e4,
                "ecols": np.ascontiguousarray(
                    e4[:, :, c * COLS_PER_CORE : (c + 1) * COLS_PER_CORE]
                ),
            }
        )
    return in_maps


def kernel(vulns, embed_table, domain_ids, _trace=False):
    if "nc_a" not in _CACHE:
        _CACHE["nc_a"] = build_kernel_a()
    if "nc_b" not in _CACHE:
        _CACHE["nc_b"] = build_kernel_b()

    res_a = run_bass_kernel_spmd(
        _CACHE["nc_a"], _prepare_a_in_maps(vulns),
        core_ids=list(range(NCORES)), trace=_trace,
    )
    _CACHE["res_a"] = res_a
    p_full = np.concatenate([res_a.results[c]["out_pm"][:, 0] for c in range(NCORES)])
    msum_full = np.concatenate(
        [res_a.results[c]["out_pm"][:, 1] for c in range(NCORES)]
    )

    res_b = run_bass_kernel_spmd(
        _CACHE["nc_b"], _prepare_b_in_maps(embed_table, domain_ids, p_full, msum_full),
        core_ids=list(range(NCORES)), trace=_trace,
    )
    _CACHE["res_b"] = res_b

    out = np.empty((S, S), dtype=np.complex64)
    for c in range(NCORES):
        r = res_b.results[c]
        sl = slice(c * COLS_PER_CORE, (c + 1) * COLS_PER_CORE)
        out[:, sl] = r["out_re"].T + 1j * r["out_im"].T
    return out


if __name__ == "__main__":
    rng = np.random.default_rng(0)
    v = rng.standard_normal((D, V), dtype=np.float32)
    et = rng.standard_normal((D, S), dtype=np.float32)
    ids = np.arange(D, dtype=np.int32)
    out = kernel(v, et, ids)
    print(out.shape, out.dtype)
